# revision 1
# baseline (speedup 1.0000x reference)
"""Trainium2 Bass kernel for the DCRF mean-field iteration module.

Math: the (B,N,N) pairwise potential is separable:
    PP[b,i,j] = g_i * g_j * (1 - u_i.u_j) * Wsym[i,j]
with g = exp(-|f|^2/2), u = f/|f| (2-component), Wsym = (W + W^T)/2.
Each mean-field step reduces sum_j PP[i,j] * v_j (v = tanh(logits/2)) to
    E_i = g_i*(S0_i) - g_i*ux_i*Sx_i - g_i*uy_i*Sy_i,
    [S0 Sx Sy] = Wsym @ [g*v, g*ux*v, g*uy*v]
i.e. one (N,N)@(N,3B) matmul per iteration instead of a 512MB tensor.

Sharding: W's columns are sharded 8 ways (64MB -> 8MB/core, SBUF-resident).
Core k owns output rows [512k, 512k+512); per iteration each core computes
its own 512x24 X-contribution, AllGathers it to (4096,24), runs 32
accumulating PE matmuls (X chunk stationary [128,24], slab moving [128,512],
float32r), and updates its own logits rows.  Host side only slices /
transposes / concatenates (sharding + unsharding).
"""

import os
import sys

import numpy as np

for _p in ("/opt/trn_rl_repo", "/root/.axon_site/_ro/trn_rl_repo"):
    if os.path.isdir(_p) and _p not in sys.path:
        sys.path.insert(0, _p)

import concourse.bass as bass  # noqa: E402
import concourse.tile as tile  # noqa: E402
from concourse import bacc  # noqa: E402
from concourse import mybir  # noqa: E402
from concourse.bass_utils import run_bass_kernel_spmd  # noqa: E402
from concourse.masks import make_identity  # noqa: E402

B = 8          # batch
G = 64         # grid
N = G * G      # 4096 nodes
ITER = 10
NCORES = 8
R = N // NCORES        # 512 own rows per core
DCH = R // 128         # 4 own 128-row blocks
NCH = N // 128         # 32 contraction chunks
S3 = 3                 # stats per node: a, bx, by
COLS = S3 * B          # 24 matmul RHS columns
WARM_MM = 20           # junk matmuls per iteration to keep PE HAM-warm

F32 = mybir.dt.float32
F32R = mybir.dt.float32r


def _build_kernel():
    nc = bacc.Bacc("TRN2", target_bir_lowering=False, debug=False,
                   num_devices=NCORES)

    # Per-core inputs (host-sharded views of the full inputs).  The W slabs
    # are declared float32r (same bits as float32; enables full-rate PE
    # matmul).  run_bass_kernel_spmd sees np.float32 for both.
    w_c = nc.dram_tensor("w_c", [N, R], F32R, kind="ExternalInput")   # W[:, own]
    w_rt = nc.dram_tensor("w_rt", [N, R], F32R, kind="ExternalInput")  # W[own, :].T
    dp_own = nc.dram_tensor("dp_own", [2, R, B], F32, kind="ExternalInput")
    lg_own = nc.dram_tensor("lg_own", [R, B], F32, kind="ExternalInput")
    out_own = nc.dram_tensor("out_own", [R, B], F32, kind="ExternalOutput")

    with tile.TileContext(nc) as tc:
        _emit(tc, nc, w_c.ap(), w_rt.ap(), dp_own.ap(), lg_own.ap(),
              out_own.ap())
    nc.compile()
    return nc


def _emit(tc, nc, w_c, w_rt, dp_own, lg_own, out_own, chain_after=None,
          comm=True):
    import contextlib

    entry = []  # input-loading instructions (for benchmark serialization)

    ctx = contextlib.ExitStack()
    with ctx:
        singles = ctx.enter_context(tc.tile_pool(name="singles", bufs=1))
        wload = ctx.enter_context(tc.tile_pool(name="wload", bufs=4))
        small = ctx.enter_context(tc.tile_pool(name="small", bufs=3))
        xpool = ctx.enter_context(tc.tile_pool(name="xpool", bufs=2))
        psum = ctx.enter_context(tc.tile_pool(name="psum", bufs=2, space="PSUM"))
        psum_t = ctx.enter_context(tc.tile_pool(name="psum_t", bufs=4, space="PSUM"))
        dram = ctx.enter_context(tc.tile_pool(name="dram", bufs=2, space="DRAM"))

        # ---- W slab: wslab[p, c, i] = (W + W^T)[p*32 + c, own_i]  (2*Wsym) ----
        # Contraction index j maps to (p, c) = (j // 32, j % 32) so every DMA
        # below is contiguous per partition.
        wslab = singles.tile([128, NCH, 512], F32R)
        w_c_r = w_c.rearrange("(p c) i -> p c i", p=128)
        w_rt_r = w_rt.rearrange("(p c) i -> p c i", p=128)
        PIECE = 4  # c-chunks per load piece (1MB per DMA)
        for q in range(NCH // PIECE):
            cs = slice(q * PIECE, (q + 1) * PIECE)
            tc_t = wload.tile([128, PIECE, 512], F32R, tag="wc")
            tr_t = wload.tile([128, PIECE, 512], F32R, tag="wr")
            entry.append(nc.sync.dma_start(out=tc_t, in_=w_c_r[:, cs, :]))
            entry.append(nc.sync.dma_start(out=tr_t, in_=w_rt_r[:, cs, :]))
            nc.vector.tensor_add(wslab[:, cs, :], tc_t, tr_t)

        # ---- per-node stats for own rows, layout [p, (d, b)], i = d*128+p ----
        fx = small.tile([128, DCH, B], F32, tag="fx", bufs=1)
        fy = small.tile([128, DCH, B], F32, tag="fy", bufs=1)
        dp_r = dp_own.rearrange("t (d p) b -> t p d b", p=128)
        entry.append(nc.sync.dma_start(out=fx, in_=dp_r[0]))
        entry.append(nc.sync.dma_start(out=fy, in_=dp_r[1]))

        sq = small.tile([128, DCH, B], F32, tag="sq", bufs=1)
        t0 = small.tile([128, DCH, B], F32, tag="t0", bufs=1)
        nc.vector.tensor_mul(sq, fx, fx)
        nc.vector.tensor_mul(t0, fy, fy)
        nc.vector.tensor_add(sq, sq, t0)
        gst = singles.tile([128, S3, DCH, B], F32)    # [g, gx, gy] (for X build)
        gcm = singles.tile([128, S3, DCH, B], F32)    # 0.5*[g, gx, gy] (combine)
        g_, gx_, gy_ = gst[:, 0], gst[:, 1], gst[:, 2]
        nc.scalar.activation(g_, sq, mybir.ActivationFunctionType.Exp, scale=-0.5)
        nrm = small.tile([128, DCH, B], F32, tag="nrm", bufs=1)
        nc.scalar.sqrt(nrm, sq)
        rin = small.tile([128, DCH, B], F32, tag="rin", bufs=1)
        nc.vector.reciprocal(rin, nrm)
        nc.vector.tensor_mul(gx_, g_, fx)
        nc.vector.tensor_mul(gx_, gx_, rin)
        nc.vector.tensor_mul(gy_, g_, fy)
        nc.vector.tensor_mul(gy_, gy_, rin)
        nc.scalar.mul(gcm, gst, 0.5)

        # ---- unary + initial v / X contribution ----
        unary = singles.tile([128, DCH, B], F32)
        entry.append(nc.sync.dma_start(
            out=unary, in_=lg_own.rearrange("(d p) b -> p d b", p=128)))

        if chain_after is not None:
            from concourse.tile_rust import add_dep_helper
            for e in entry:
                add_dep_helper(e.ins, chain_after.ins,
                               reason="bench serialization")

        ident = singles.tile([128, 128], F32)
        make_identity(nc, ident)

        def build_xc(v_t):
            # xc[p, d, s, b] = gst[p, s, d, b] * v[p, d, b]  (f32r out so the
            # bounce store can use HWDGE without a cast)
            xc_t = small.tile([128, DCH, S3, B], F32R, tag="xc")
            for s in range(S3):
                nc.vector.tensor_mul(xc_t[:, :, s, :], gst[:, s], v_t)
            return xc_t

        v0 = small.tile([128, DCH, B], F32, tag="v")
        nc.scalar.activation(v0, unary, mybir.ActivationFunctionType.Tanh,
                             scale=0.5)
        xc = build_xc(v0)

        lgt = None
        for it in range(ITER):
            # own X contribution -> DRAM bounce (R, COLS), row i = d*128+p
            bounce_in = dram.tile([R, COLS], F32R, tag="bin")
            nc.sync.dma_start(
                out=bounce_in.rearrange("(d p) c -> p d c", p=128), in_=xc)
            bounce_out = dram.tile([N, COLS], F32R, tag="bout")
            if comm:
                nc.gpsimd.collective_compute(
                    "AllGather",
                    mybir.AluOpType.bypass,
                    replica_groups=[list(range(NCORES))],
                    ins=[bounce_in.opt()],
                    outs=[bounce_out.opt()],
                )
            else:
                # single-core timing proxy: local copy instead of AllGather
                nc.sync.dma_start(out=bounce_out[0:R, :], in_=bounce_in)
            # X[p, c, col] = bounce_out[p*32 + c, col]; two halves so the
            # matmul chain starts as soon as chunks 0-15 land
            xall = xpool.tile([128, NCH, COLS], F32R, tag="xall")
            bo_r = bounce_out.rearrange("(p c) k -> p c k", p=128)
            H = NCH // 2
            nc.sync.dma_start(out=xall[:, :H], in_=bo_r[:, :H])
            nc.sync.dma_start(out=xall[:, H:], in_=bo_r[:, H:])

            # y[n, i] = sum_j X[j, n] * wslab[j, i]  (j = p*32+c)
            y_ps = psum.tile([COLS, 512], F32, tag="yps")
            for c in range(NCH):
                nc.tensor.matmul(y_ps, lhsT=xall[:, c, :], rhs=wslab[:, c, :],
                                 start=(c == 0), stop=(c == NCH - 1))

            # transpose to [p, (d, col)] via PE  (copy on DVE: ~2-9x faster
            # than ACT's table-based Copy, and this sits on the critical path)
            y_sb = small.tile([COLS, 512], F32, tag="ysb")
            nc.vector.tensor_copy(y_sb, y_ps)
            yt = small.tile([128, DCH, S3, B], F32, tag="yt")
            for d in range(DCH):
                tp = psum_t.tile([128, COLS], F32, tag="tp")
                nc.tensor.transpose(tp, y_sb[:, d * 128:(d + 1) * 128],
                                    ident[:COLS, :COLS])
                nc.vector.tensor_copy(yt[:, d], tp)

            if it < ITER - 1:
                # keep the PE HAM-warm through the AllGather gap: junk
                # matmuls on already-resident data into a scratch PSUM bank
                warm_ps = psum.tile([COLS, 512], F32, tag="warm")
                for wi in range(WARM_MM):
                    nc.tensor.matmul(warm_ps, lhsT=xall[:, wi % NCH, :],
                                     rhs=wslab[:, wi % NCH, :],
                                     start=True, stop=True)

            # E = 0.5*(g*S0 - gx*Sx - gy*Sy);  logits = unary + E
            e_t = small.tile([128, DCH, B], F32, tag="e")
            t1 = small.tile([128, DCH, B], F32, tag="t1")
            nc.vector.tensor_mul(e_t, gcm[:, 0], yt[:, :, 0, :])
            nc.vector.tensor_mul(t1, gcm[:, 1], yt[:, :, 1, :])
            nc.vector.tensor_sub(e_t, e_t, t1)
            nc.vector.tensor_mul(t1, gcm[:, 2], yt[:, :, 2, :])
            nc.vector.tensor_sub(e_t, e_t, t1)
            lgt = small.tile([128, DCH, B], F32, tag="lgt")
            nc.vector.tensor_add(lgt, unary, e_t)

            if it < ITER - 1:
                v_t = small.tile([128, DCH, B], F32, tag="v")
                nc.scalar.activation(v_t, lgt,
                                     mybir.ActivationFunctionType.Tanh,
                                     scale=0.5)
                xc = build_xc(v_t)

        return nc.sync.dma_start(
            out=out_own.rearrange("(d p) b -> p d b", p=128), in_=lgt)


_NC_CACHE = None


def _get_nc():
    global _NC_CACHE
    if _NC_CACHE is None:
        _NC_CACHE = _build_kernel()
    return _NC_CACHE


def kernel(delta_p, logits, W, _trace=False):
    delta_p = np.ascontiguousarray(np.asarray(delta_p, dtype=np.float32))
    logits = np.ascontiguousarray(np.asarray(logits, dtype=np.float32))
    W = np.ascontiguousarray(np.asarray(W, dtype=np.float32))

    feats = delta_p.reshape(B, N, 2)
    in_maps = []
    for k in range(NCORES):
        rows = slice(R * k, R * (k + 1))
        in_maps.append({
            "w_c": np.ascontiguousarray(W[0][:, rows]),
            "w_rt": np.ascontiguousarray(W[0][rows, :].T),
            # (2, R, B): [component, own row, batch]
            "dp_own": np.ascontiguousarray(feats[:, rows, :].transpose(2, 1, 0)),
            "lg_own": np.ascontiguousarray(logits[:, rows, 0].T),
        })

    res = run_bass_kernel_spmd(_get_nc(), in_maps, core_ids=list(range(NCORES)),
                               trace=_trace)
    # out_own per core: (R, B) -> full (B, N, 1)
    out = np.empty((B, N, 1), dtype=np.float32)
    for k, r in enumerate(res.results):
        out[:, R * k:R * (k + 1), 0] = r["out_own"].T
    if _trace:
        kernel._last_result = res
    return out

